# revision 1
# baseline (speedup 1.0000x reference)
"""Trainium2 Bass kernel for nn_DecoderRNN (LSTM decoder + 32k-vocab projection).

Distribution over 8 NeuronCores:
  - The 23-step LSTM recurrence (B=64) is replicated on every core. Per step a
    single fused fp16 weight stream [w | cS | hS | cU | hU | ones] (1562 rows,
    even-aligned blocks) x [lstmS gates | lstmU gates] (2600 cols) accumulates
    in fp32 PSUM; biases ride the ones-rows.
  - The vocab projection (61 GFLOP) is sharded 4-way over vocab x 2-way over
    batch: each core keeps a [626, 8000] bf16 fcW slice (incl. fcb row)
    resident in SBUF and computes its [24*32, 626] @ [626, 8000] tile.
  - make_ft (32 batched 25x25 matmuls/step per core) builds per-batch
    stationary tiles via DRAM-bounce strided DMAs.
  - Batch-half selection is done on the host by rotating the batch order per
    core, so the device program is SPMD-identical.
"""

import numpy as np
import ml_dtypes

B, T, V, E, D = 64, 24, 32000, 256, 25
H = D * D                       # 625
GCOLS = 4 * H + 4 * D           # 2600
NCORES = 8
NB, NV = 2, 4                   # batch shards x vocab shards
BL, VL = B // NB, V // NV       # 32, 8000
NSTEP = T - 1                   # 23
F16, BF16 = np.float16, ml_dtypes.bfloat16

# x-row layout: every block 32-aligned within its 128-chunk (pad rows carry
# zero weights, pad state columns are zeroed)
XW = 0            # 256 word rows
XCS = 256         # 625 cS rows, padded to 640
XHS = 896         # 625 hS rows, padded to 640
XCU = 1536        # 25 cU rows, padded to 32
XHU = 1568        # 25 hU rows, padded to 32
XONE = 1600       # 2 ones rows
XROWS = 1602
NCHUNK = 13       # 12*128 + 66

_COMPILED = {}


def _chunk_rows(c):
    return 128 if c < NCHUNK - 1 else XROWS - 128 * (NCHUNK - 1)


def _win(start, end):
    out, p = [], start
    while p < end:
        n = min(end - p, 128 * (p // 128 + 1) - p)
        out.append((p, n))
        p += n
    return out


def _build_program():
    import concourse.bass as bass
    import concourse.tile as tile
    from concourse import bacc, mybir
    from contextlib import ExitStack

    f16, f32 = mybir.dt.float16, mybir.dt.float32
    bf16, i32 = mybir.dt.bfloat16, mybir.dt.int32
    AF = mybir.ActivationFunctionType

    nc = bacc.Bacc("TRN2", target_bir_lowering=False, debug=False,
                   num_devices=NCORES)

    w_all = nc.dram_tensor("w_all", [XROWS, GCOLS], f16, kind="ExternalInput").ap()
    fcq = nc.dram_tensor("fcq", [H + 1, VL], bf16, kind="ExternalInput").ap()
    wut = nc.dram_tensor("wut", [34, H], f16, kind="ExternalInput").ap()
    feat = nc.dram_tensor("feat", [E + 1, B], f16, kind="ExternalInput").ap()
    szt = nc.dram_tensor("szt", [E + 1, H], f16, kind="ExternalInput").ap()
    emb16 = nc.dram_tensor("emb16", [V, E], f16, kind="ExternalInput").ap()
    gidx = nc.dram_tensor("gidx", [1536], i32, kind="ExternalInput").ap()
    iden = nc.dram_tensor("iden", [128, 128], f32, kind="ExternalInput").ap()
    ones16 = nc.dram_tensor("ones16", [2, 64], f16, kind="ExternalInput").ap()
    onesbf = nc.dram_tensor("onesbf", [1, 768], bf16, kind="ExternalInput").ap()
    zerbf = nc.dram_tensor("zerbf", [126, 32], bf16, kind="ExternalInput").ap()
    out = nc.dram_tensor("out", [T, BL, VL], f32, kind="ExternalOutput").ap()

    with tile.TileContext(nc) as tc, ExitStack() as ctx:
        const = ctx.enter_context(tc.tile_pool(name="const", bufs=1))
        states = ctx.enter_context(tc.tile_pool(name="states", bufs=2))
        gact = ctx.enter_context(tc.tile_pool(name="gact", bufs=1))
        xts_pool = ctx.enter_context(tc.tile_pool(name="xtsp", bufs=2))
        ft_pool = ctx.enter_context(tc.tile_pool(name="ftp", bufs=1))
        lout = ctx.enter_context(tc.tile_pool(name="loutp", bufs=2))
        dram = ctx.enter_context(tc.tile_pool(name="dram", bufs=2, space="DRAM"))
        gpsum = ctx.enter_context(tc.tile_pool(name="gpsum", bufs=3, space="PSUM"))
        tpsum = ctx.enter_context(tc.tile_pool(name="tpsum", bufs=1, space="PSUM"))
        ttpsum = ctx.enter_context(tc.tile_pool(name="ttps", bufs=2, space="PSUM"))
        fcpsum = ctx.enter_context(tc.tile_pool(name="fcps", bufs=2, space="PSUM"))

        # ---------- persistent SBUF ----------
        WA = []
        for c in range(NCHUNK):
            r = _chunk_rows(c)
            t_ = const.tile([r, GCOLS], f16, tag=f"wa{c}")
            nc.sync.dma_start(t_[:], w_all[128 * c:128 * c + r, :])
            WA.append(t_)
        FC = []
        for c in range(5):
            r = 126 if c == 4 else 125
            t_ = const.tile([r, VL], bf16, tag=f"fc{c}")
            nc.sync.dma_start(t_[:], fcq[125 * c:125 * c + r, :])
            FC.append(t_)
        WUT = const.tile([34, H], f16, tag="wut")
        nc.sync.dma_start(WUT[:], wut)
        IDEN = const.tile([128, 128], f32, tag="iden")
        nc.sync.dma_start(IDEN[:], iden)
        ONES16 = const.tile([2, 64], f16, tag="ones16")
        nc.sync.dma_start(ONES16[:], ones16)
        F32Z = const.tile([64, 16], f32, tag="f32z")
        nc.vector.memset(F32Z[:], 0.0)
        FEAT, SZT = [], []
        for c, r in ((0, 128), (1, 128), (2, 1)):
            tf = const.tile([r, B], f16, tag=f"feat{c}")
            nc.sync.dma_start(tf[:], feat[128 * c:128 * c + r, :])
            FEAT.append(tf)
            ts = const.tile([r, H], f16, tag=f"szt{c}")
            nc.sync.dma_start(ts[:], szt[128 * c:128 * c + r, :])
            SZT.append(ts)

        WT0 = const.tile([128, 1536], f16, tag="wt0")
        WT1 = const.tile([128, 1536], f16, tag="wt1")
        IDX = const.tile([128, 12], i32, tag="idx")
        nc.sync.dma_start(IDX[:], gidx.rearrange("(g p) -> p g", p=128))
        for g in range(12):
            gb = const.tile([128, E], f16, tag="gbuf")
            nc.gpsimd.indirect_dma_start(
                gb[:], None, emb16[0:128, :],
                bass.IndirectOffsetOnAxis(ap=IDX[:, g:g + 1], axis=0))
            nc.sync.dma_start_transpose(WT0[:, 128 * g:128 * g + 128],
                                        gb[:, 0:128])
            nc.sync.dma_start_transpose(WT1[:, 128 * g:128 * g + 128],
                                        gb[:, 128:256])

        WO = []
        for c in range(5):
            r = 126 if c == 4 else 125
            t_ = const.tile([r, T * BL], bf16, tag=f"wo{c}")
            nc.sync.dma_start(t_[:, 0:BL], zerbf[0:r, :])
            WO.append(t_)
        nc.sync.dma_start(WO[4][125:126, :], onesbf)

        # ---------- helpers ----------
        def xts_ap(xts, xr0, n):
            c = xr0 // 128
            assert (xr0 + n - 1) // 128 == c and c >= 2
            return xts[xr0 % 128:xr0 % 128 + n,
                       64 * (c - 2):64 * (c - 2) + 64]

        def transpose_into(src, xts, xr0, total):
            col = 0
            for (p0, n) in _win(xr0, xr0 + total):
                ps = tpsum.tile([128, B], f32, tag="tp")
                nc.tensor.transpose(ps[0:n, :], src[:, col:col + n],
                                    IDEN[0:B, 0:B])
                nc.vector.tensor_copy(xts_ap(xts, p0, n), ps[0:n, :])
                col += n

        BANKS = [(0, 500), (500, 500), (1000, 500), (1500, 500),
                 (2000, 500), (2500, 100)]
        ACT_JOBS = [(0, 625, AF.Sigmoid, "si"), (625, 1250, AF.Sigmoid, "sf"),
                    (1250, 1875, AF.Tanh, "sg"), (1875, 2500, AF.Sigmoid, "so"),
                    (2500, 2525, AF.Sigmoid, "siu"),
                    (2525, 2550, AF.Sigmoid, "sfu"),
                    (2550, 2575, AF.Tanh, "sgu"),
                    (2575, 2600, AF.Sigmoid, "sou")]

        def fc_mchunk(m):
            for n in range(VL // 500):
                fp = fcpsum.tile([128, 500], f32, tag="fcp")
                for c in range(5):
                    for tq in range(4):
                        tt_ = 4 * m + tq
                        nc.tensor.matmul(
                            fp[32 * tq:32 * tq + 32, :],
                            WO[c][:, BL * tt_:BL * tt_ + BL],
                            FC[c][:, 500 * n:500 * n + 500],
                            start=(c == 0), stop=(c == 4),
                            tile_position=(0, 32 * tq))
                lo = lout.tile([128, 500], f32, tag="lo")
                nc.vector.tensor_copy(lo[:], fp[:])
                nc.sync.dma_start(
                    out[4 * m:4 * m + 4, :, 500 * n:500 * n + 500], lo[:])

        # ---------- initial state ----------
        CS = states.tile([B, 640], f32, tag="cs")
        sp0 = gpsum.tile([B, 500], f32, tag="gp")
        sp1 = gpsum.tile([B, 500], f32, tag="gp")
        for ci in range(3):
            nc.tensor.matmul(sp0[:, 0:500], FEAT[ci][:], SZT[ci][:, 0:500],
                             start=(ci == 0), stop=(ci == 2))
            nc.tensor.matmul(sp1[:, 0:125], FEAT[ci][:], SZT[ci][:, 500:625],
                             start=(ci == 0), stop=(ci == 2))
        nc.vector.tensor_copy(CS[:, 0:500], sp0[:, 0:500])
        nc.vector.tensor_copy(CS[:, 500:625], sp1[:, 0:125])
        nc.vector.tensor_copy(CS[:, 625:640], F32Z[:, 0:15])
        CU = states.tile([B, 32], f32, tag="cu")
        nc.vector.memset(CU[:], 0.0)

        XTS = xts_pool.tile([128, 11 * 64], f16, tag="xts")
        nc.vector.memset(XTS[:], 0.0)
        nc.vector.tensor_copy(xts_ap(XTS, XONE, 2), ONES16[:])
        transpose_into(CS, XTS, XCS, 640)

        # ---------- recurrence ----------
        for s in range(NSTEP):
            def xchunk(c, _xts=XTS, _s=s):
                if c == 0:
                    return WT0[:, 64 * _s:64 * _s + 64]
                if c == 1:
                    return WT1[:, 64 * _s:64 * _s + 64]
                r = _chunk_rows(c)
                return _xts[0:r, 64 * (c - 2):64 * (c - 2) + 64]

            gps = []
            for (col0, n) in BANKS:
                gp = gpsum.tile([B, 500], f32, tag="gp")
                for c in range(NCHUNK):
                    nc.tensor.matmul(gp[:, 0:n], xchunk(c),
                                     WA[c][:, col0:col0 + n],
                                     start=(c == 0), stop=(c == NCHUNK - 1))
                gps.append(gp)

            gt = {}
            for key, w in (("si", H), ("sf", H), ("sg", H), ("so", H),
                           ("siu", D), ("sfu", D), ("sgu", D), ("sou", D)):
                gt[key] = gact.tile([B, w], f32, tag=key, name=key)
            for (g0, g1, fn, key) in ACT_JOBS:
                done = 0
                while done < g1 - g0:
                    gcol = g0 + done
                    bk, bcol = gcol // 500, gcol % 500
                    n = min(g1 - gcol, 500 - bcol)
                    nc.scalar.activation(gt[key][:, done:done + n],
                                         gps[bk][:, bcol:bcol + n], fn)
                    done += n

            CSn = states.tile([B, 640], f32, tag="cs")
            t1 = gact.tile([B, H], f32, tag="t1")
            t2 = gact.tile([B, H], f32, tag="t2")
            nc.vector.tensor_mul(t1[:], gt["sf"][:], CS[:, 0:H])
            nc.vector.tensor_mul(t2[:], gt["si"][:], gt["sg"][:])
            nc.vector.tensor_add(CSn[:, 0:H], t1[:], t2[:])
            nc.vector.tensor_copy(CSn[:, H:640], F32Z[:, 0:15])
            TC = gact.tile([B, H], f32, tag="tc")
            nc.scalar.activation(TC[:], CSn[:, 0:H], AF.Tanh)
            HSn = states.tile([B, 640], f32, tag="hs")
            nc.vector.tensor_mul(HSn[:, 0:H], gt["so"][:], TC[:])
            nc.vector.tensor_copy(HSn[:, H:640], F32Z[:, 0:15])

            CUn = states.tile([B, 32], f32, tag="cu")
            t1u = gact.tile([B, D], f32, tag="t1u")
            t2u = gact.tile([B, D], f32, tag="t2u")
            nc.vector.tensor_mul(t1u[:], gt["sfu"][:], CU[:, 0:D])
            nc.vector.tensor_mul(t2u[:], gt["siu"][:], gt["sgu"][:])
            nc.vector.tensor_add(CUn[:, 0:D], t1u[:], t2u[:])
            nc.vector.tensor_copy(CUn[:, D:32], F32Z[:, 0:7])
            TCU = gact.tile([B, D], f32, tag="tcu")
            nc.scalar.activation(TCU[:], CUn[:, 0:D], AF.Tanh)
            HUn = states.tile([B, 32], f32, tag="hu")
            nc.vector.tensor_mul(HUn[:, 0:D], gt["sou"][:], TCU[:])
            nc.vector.tensor_copy(HUn[:, D:32], F32Z[:, 0:7])

            if s < NSTEP - 1:
                XTSn = xts_pool.tile([128, 11 * 64], f16, tag="xts")
                nc.vector.tensor_copy(xts_ap(XTSn, XONE, 2), ONES16[:])
                transpose_into(CUn, XTSn, XCU, 32)
                transpose_into(CSn, XTSn, XCS, 640)
                transpose_into(HSn, XTSn, XHS, 640)
            else:
                XTSn = None
            hups = tpsum.tile([128, B], f32, tag="tp")
            nc.tensor.transpose(hups[0:32, :], HUn[:], IDEN[0:B, 0:B])
            HUTn = states.tile([34, B], f16, tag="hut")
            nc.vector.tensor_copy(HUTn[0:32, :], hups[0:32, :])
            nc.vector.tensor_copy(HUTn[32:34, :], ONES16[:])
            if XTSn is not None:
                nc.vector.tensor_copy(xts_ap(XTSn, XHU, 32), hups[0:32, :])

            up0 = gpsum.tile([B, 500], f32, tag="gp")
            up1 = gpsum.tile([B, 500], f32, tag="gp")
            nc.tensor.matmul(up0[:, 0:500], HUTn[:], WUT[:, 0:500],
                             start=True, stop=True)
            nc.tensor.matmul(up1[:, 0:125], HUTn[:], WUT[:, 500:625],
                             start=True, stop=True)
            UT = ft_pool.tile([B, H], f16, tag="ut")
            nc.vector.tensor_copy(UT[:, 0:500], up0[:, 0:500])
            nc.vector.tensor_copy(UT[:, 500:625], up1[:, 0:125])

            # make_ft for local batch rows 0..31 via DRAM-bounce regroup
            utd = dram.tile([BL, H], f16, tag="utd")
            nc.sync.dma_start(utd[:], UT[0:BL, :])
            m2d = dram.tile([BL, H], f16, tag="m2d")
            nc.gpsimd.dma_start(m2d[:], HSn[0:BL, 0:H])   # f32 -> f16 cast
            UTT = ft_pool.tile([D, BL * D], f16, tag="utt")
            nc.sync.dma_start(
                UTT[:], bass.AP(utd.tensor, 0, [[D, D], [H, BL], [1, D]]))
            M2T = ft_pool.tile([D, BL * D], f16, tag="m2t")
            nc.sync.dma_start(
                M2T[:], bass.AP(m2d.tensor, 0, [[D, D], [H, BL], [1, D]]))
            TTS = ft_pool.tile([D, BL * D], f32, tag="tts")
            for half in range(2):
                tt = ttpsum.tile([D, 16 * D], f32, tag="tt")
                for bl in range(16):
                    b = 16 * half + bl
                    nc.tensor.matmul(tt[:, D * bl:D * bl + D],
                                     M2T[:, D * b:D * b + D],
                                     UTT[:, D * b:D * b + D],
                                     start=True, stop=True)
                nc.vector.tensor_copy(
                    TTS[:].rearrange("j (i b) -> j i b", i=D)
                       [:, :, 16 * half:16 * half + 16],
                    tt[:].rearrange("j (b i) -> j i b", b=16))
            ttd = dram.tile([D, BL * D], f32, tag="ttd")
            nc.sync.dma_start(ttd[:], TTS[:])
            tcol = s + 1
            for c in range(5):
                nc.gpsimd.dma_start(
                    WO[c][0:125, BL * tcol:BL * tcol + BL],
                    bass.AP(ttd.tensor, 5 * c * BL * D,
                            [[BL * D, 5], [BL, D], [1, BL]]))

            CS, CU, XTS = CSn, CUn, XTSn

            if s >= 2 and (s - 2) % 4 == 0:
                fc_mchunk((s - 2) // 4)

    nc.compile()
    return nc


def _host_prep(inputs):
    f32 = lambda k: np.asarray(inputs[k], dtype=np.float32)
    features = f32("features")
    captions = np.asarray(inputs["captions"]).astype(np.int64)
    embed = f32("embed_table")
    WihS, WhhS = f32("WihS"), f32("WhhS")
    bihS, bhhS = f32("bihS"), f32("bhhS")
    WihU, WhhU = f32("WihU"), f32("WhhU")
    bihU, bhhU = f32("bihU"), f32("bhhU")
    fcW, fcb = f32("fcW"), f32("fcb")
    szW, szb = f32("szW"), f32("szb")
    wuW, wub = f32("wuW"), f32("wub")

    w_all = np.zeros((XROWS, GCOLS), np.float32)
    WihS_T, WihU_T = WihS.T, WihU.T
    w_all[XW:XW + 256, :2500] = WihS_T[25:281]
    w_all[XW:XW + 256, 2500:] = WihU_T[25:281]
    w_all[XCS:XCS + 625, :2500] = WihS_T[281:906]
    w_all[XCS:XCS + 625, 2500:] = WihU_T[281:906]
    w_all[XHS:XHS + 625, :2500] = WhhS.T
    w_all[XCU:XCU + 25, :2500] = WihS_T[0:25]
    w_all[XCU:XCU + 25, 2500:] = WihU_T[0:25]
    w_all[XHU:XHU + 25, 2500:] = WhhU.T
    w_all[XONE, :2500] = bihS + bhhS
    w_all[XONE, 2500:] = bihU + bhhU
    w_all = np.ascontiguousarray(w_all).astype(F16)

    fcW_perm = np.ascontiguousarray(
        fcW.reshape(V, D, D).transpose(0, 2, 1).reshape(V, H))
    wuW_perm = np.ascontiguousarray(
        wuW.reshape(D, D, D).transpose(1, 0, 2).reshape(H, D))
    wub_perm = np.ascontiguousarray(wub.reshape(D, D).T.reshape(H))
    wut = np.zeros((34, H), np.float32)
    wut[0:25] = wuW_perm.T
    wut[32] = wub_perm
    wut = wut.astype(F16)

    szt = np.concatenate([szW.T, szb[None, :]], 0).astype(F16)
    emb16 = embed.astype(F16)
    iden = np.eye(128, dtype=np.float32)

    in_maps = []
    for bh in range(NB):
        rot = np.roll(np.arange(B), -BL * bh)
        feat_r = np.concatenate([features[rot].T,
                                 np.ones((1, B), np.float32)], 0).astype(F16)
        gidx = np.zeros(1536, np.int32)
        cap_r = captions[rot]
        for s in range(1, 23):
            gidx[64 * s:64 * s + 64] = cap_r[:, s]
        for vq in range(NV):
            fcq = np.concatenate(
                [fcW_perm.T[:, VL * vq:VL * vq + VL],
                 fcb[None, VL * vq:VL * vq + VL]], 0).astype(BF16)
            in_maps.append({
                "w_all": w_all, "fcq": np.ascontiguousarray(fcq),
                "wut": wut, "feat": feat_r, "szt": szt,
                "emb16": emb16, "gidx": gidx, "iden": iden,
                "ones16": np.ones((2, 64), F16),
                "onesbf": np.ones((1, 768), BF16),
                "zerbf": np.zeros((126, 32), BF16),
            })
    return in_maps


def kernel(**inputs):
    from concourse.bass_utils import run_bass_kernel_spmd

    if "prog" not in _COMPILED:
        _COMPILED["prog"] = _build_program()
    nc = _COMPILED["prog"]

    in_maps = _host_prep(inputs)
    res = run_bass_kernel_spmd(nc, in_maps, list(range(NCORES)))

    out = np.zeros((T, B, 1, V), np.float32)
    ci = 0
    for bh in range(NB):
        for vq in range(NV):
            o = np.asarray(res.results[ci]["out"])
            out[:, BL * bh:BL * bh + BL, 0, VL * vq:VL * vq + VL] = o
            ci += 1
    return out



# revision 2
# speedup vs baseline: 1.0264x; 1.0264x over previous
"""Trainium2 Bass kernel for nn_DecoderRNN (LSTM decoder + 32k-vocab projection), v2.

Distribution over 8 NeuronCores (unchanged from v1): recurrence replicated,
vocab projection sharded 4-way over vocab x 2-way over batch.

v2 performance redesign (v1 measured 1.04 ms, PE-bound at half clock):
  - Gates GEMM [64,1602]x[1602,2600] runs as 2-way column-tiled matmul pairs:
    gate columns are host-permuted into a "lo" half (A, PE cols 0-63 -> PSUM
    partitions 0-63) and "hi" half (B, PE cols 64-127 -> partitions 64-127),
    so both halves stream concurrently - 2x PE throughput at M=64.
  - States/gates live in a folded [128, 320] layout (batch x H-half stacked
    across partitions), halving DVE/ACT elementwise time and keeping every
    ACT/DVE op partition-aligned.
  - State transposes into the x^T layout run on TensorE into fp16 PSUM
    (straddle blocks via tile_position), evacuated with one wide DVE copy
    per state; the gate stream is chunk-major so the next step's matmuls
    reach the cS-dependent chunks only after the transposes land.
  - make_ft's small matmuls (and the WO writeback chain) are deferred by one
    step so they ride behind the next step's gate stream instead of stalling
    the PE on DMA latency.
  - Everything recurrent is fp16 (PSUM accumulation stays fp32).
  - make_ft's 32 25x25 matmuls run 4-way column-packed via tile_position.
  - fc projection streams [125,128]x[125,500] fp16 single matmuls (M=128),
    bank-major, with fp16 PSUM evacuation split across DVE/ACT and fp16
    output DMA. The t=0 logits row (= fcb) is computed on the host.
"""

import numpy as np

B, T, V, E, D = 64, 24, 32000, 256, 25
H = D * D                       # 625
GCOLS = 2660                    # 2x (4x320) + 100 u-gate cols
NCORES = 8
NB, NV = 2, 4                   # batch shards x vocab shards
BL, VL = B // NB, V // NV       # 32, 8000
NSTEP = T - 1                   # 23
F16 = np.float16

LO_W = 320                      # folded lo-half width (h 0..319)
HI_W = 305                      # hi-half width (h 320..624)

# x-row layout (identical to v1): word 0-255, cS 256-895 (625+15 pad),
# hS 896-1535, chunk 12: cU(25+7) hU(25+7) ones(2)
XROWS = 1602
NCHUNK = 13

# A/B gate-column split (host-permuted), gate order [g, i, f, o] with the
# hi halves zero-padded to 320 so both PSUM halves share column offsets:
# A (cols 0-1280):    g_lo i_lo f_lo o_lo  (each 320)
# B (cols 1280-2660): g_hi i_hi f_hi o_hi (305+15 pad each), iu fu ou gu (100)
BANK_A = [(0, 512), (512, 1024), (1024, 1280)]
BANK_B = [(1280, 1792), (1792, 2304), (2304, 2660)]

_COMPILED = {}


def _chunk_rows(c):
    return 128 if c < NCHUNK - 1 else XROWS - 128 * (NCHUNK - 1)


def _build_program():
    import concourse.bass as bass
    import concourse.tile as tile
    from concourse import bacc, mybir
    from contextlib import ExitStack

    f16, f32 = mybir.dt.float16, mybir.dt.float32
    AF = mybir.ActivationFunctionType

    nc = bacc.Bacc("TRN2", target_bir_lowering=False, debug=False,
                   num_devices=NCORES)

    w_all = nc.dram_tensor("w_all", [XROWS, GCOLS], f16, kind="ExternalInput").ap()
    fcq = nc.dram_tensor("fcq", [H + 1, VL], f16, kind="ExternalInput").ap()
    wut = nc.dram_tensor("wut", [34, H], f16, kind="ExternalInput").ap()
    feat = nc.dram_tensor("feat", [E + 1, B], f16, kind="ExternalInput").ap()
    szt = nc.dram_tensor("szt", [E + 1, H], f16, kind="ExternalInput").ap()
    wtin = nc.dram_tensor("wtin", [E, 64 * NSTEP], f16, kind="ExternalInput").ap()
    onesf = nc.dram_tensor("onesf", [1, T * BL], f16, kind="ExternalInput").ap()
    iden = nc.dram_tensor("iden", [128, 128], f16, kind="ExternalInput").ap()
    out = nc.dram_tensor("out", [T, BL, VL], f16, kind="ExternalOutput").ap()

    with tile.TileContext(nc) as tc, ExitStack() as ctx:
        const = ctx.enter_context(tc.tile_pool(name="const", bufs=1))
        gact = ctx.enter_context(tc.tile_pool(name="gact", bufs=1))
        states = ctx.enter_context(tc.tile_pool(name="states", bufs=2))
        xts_pool = ctx.enter_context(tc.tile_pool(name="xtsp", bufs=2))
        ft_pool = ctx.enter_context(tc.tile_pool(name="ftp", bufs=2))
        lout = ctx.enter_context(tc.tile_pool(name="loutp", bufs=3))
        dram = ctx.enter_context(tc.tile_pool(name="dram", bufs=2, space="DRAM"))
        gpsA = ctx.enter_context(tc.tile_pool(name="gpsA", bufs=2, space="PSUM"))
        gpsB = ctx.enter_context(tc.tile_pool(name="gpsB", bufs=2, space="PSUM"))
        gpsC = ctx.enter_context(tc.tile_pool(name="gpsC", bufs=2, space="PSUM"))
        fcps = ctx.enter_context(tc.tile_pool(name="fcps", bufs=2, space="PSUM"))

        # ---------- persistent SBUF ----------
        WA = []
        for c in range(NCHUNK):
            r = _chunk_rows(c)
            t_ = const.tile([r, GCOLS], f16, tag=f"wa{c}")
            nc.sync.dma_start(t_[:], w_all[128 * c:128 * c + r, :])
            WA.append(t_)
        FC = []
        for c in range(5):
            r = 126 if c == 4 else 125
            t_ = const.tile([r, VL], f16, tag=f"fc{c}")
            nc.sync.dma_start(t_[:], fcq[125 * c:125 * c + r, :])
            FC.append(t_)
        WUT = const.tile([34, H], f16, tag="wut")
        nc.sync.dma_start(WUT[:], wut)
        FEAT, SZT = [], []
        for c, r in ((0, 128), (1, 128), (2, 1)):
            tf = const.tile([r, B], f16, tag=f"feat{c}")
            nc.sync.dma_start(tf[:], feat[128 * c:128 * c + r, :])
            FEAT.append(tf)
            ts = const.tile([r, H], f16, tag=f"szt{c}")
            nc.sync.dma_start(ts[:], szt[128 * c:128 * c + r, :])
            SZT.append(ts)
        WT0 = const.tile([128, 64 * NSTEP], f16, tag="wt0")
        nc.sync.dma_start(WT0[:], wtin[0:128, :])
        WT1 = const.tile([128, 64 * NSTEP], f16, tag="wt1")
        nc.sync.dma_start(WT1[:], wtin[128:256, :])
        WO = []
        for c in range(5):
            r = 126 if c == 4 else 125
            t_ = const.tile([r, T * BL], f16, tag=f"wo{c}")
            WO.append(t_)
        nc.sync.dma_start(WO[4][125:126, :], onesf)
        IDEN = const.tile([128, 128], f16, tag="iden")
        nc.sync.dma_start(IDEN[:], iden)
        ONES = const.tile([128, 64], f16, tag="ones")
        nc.vector.memset(ONES[:], 1.0)

        # gate tile: [g | i | f | o] x 320 cols, lo half on partitions 0-63,
        # hi half (305 + 15 exact-zero pad) on 64-127
        GT = gact.tile([128, 4 * LO_W], f16, tag="gt")
        GUS = gact.tile([128, 75], f16, tag="gus")
        GUT = gact.tile([128, 25], f16, tag="gut")
        TC = gact.tile([128, LO_W], f16, tag="tc")
        TCU = gact.tile([128, 25], f16, tag="tcu")
        T1 = gact.tile([128, LO_W], f16, tag="t1")
        T2 = gact.tile([128, LO_W], f16, tag="t2")
        T1U = gact.tile([128, 25], f16, tag="t1u")
        T2U = gact.tile([128, 25], f16, tag="t2u")
        HUT = gact.tile([34, B], f16, tag="hut")
        nc.vector.memset(HUT[32:34, :], 1.0)  # bias rows (row 33 unused)

        # ---------- helpers ----------
        ILO = IDEN[0:64, 0:64]     # identity at partitions 0-63
        IHI = IDEN[64:128, 64:128]  # identity at partitions 64-127

        def emit_transposes(cs16, hs16, cun, hun):
            """Build next-step x^T tile from folded fp16 states on TensorE."""
            xn = xts_pool.tile([128, 11 * 64], f16, tag="xts")
            tmm = nc.tensor.matmul

            def state_t(src, blk0):
                tp = fcps.tile([128, 448], f16, tag="fp")
                tmm(tp[:, 0:64], src[0:64, 0:128], ILO,
                    is_transpose=True, tile_position=(0, 0))
                tmm(tp[:, 64:128], src[0:64, 128:256], ILO,
                    is_transpose=True, tile_position=(0, 0))
                tmm(tp[0:64, 128:192], src[0:64, 256:320], ILO,
                    is_transpose=True, tile_position=(0, 0))
                tmm(tp[64:128, 128:192], src[64:128, 0:64], IHI,
                    is_transpose=True, tile_position=(64, 64))
                tmm(tp[:, 192:256], src[64:128, 64:192], IHI,
                    is_transpose=True, tile_position=(64, 0))
                tmm(tp[:, 256:320], src[64:128, 192:320], IHI,
                    is_transpose=True, tile_position=(64, 0))
                return tp

            tpc = state_t(cs16, 0)
            tph = state_t(hs16, 5)
            # u block into spare cols of tph's bank (cun/hun carry zero pads)
            tmm(tph[0:32, 320:384], cun[64:128, 0:32], IHI,
                is_transpose=True, tile_position=(64, 0))
            tmm(tph[32:64, 320:384], hun[64:128, 0:32], IHI,
                is_transpose=True, tile_position=(64, 32))
            nc.vector.tensor_copy(xn[:, 0:320], tpc[:, 0:320])
            nc.vector.tensor_copy(xn[:, 320:640], tph[:, 0:320])
            nc.vector.tensor_copy(xn[0:64, 640:704], tph[0:64, 320:384])
            nc.vector.tensor_copy(xn[64:66, 640:704], ONES[64:66, :])
            return xn

        def fc_mchunk(m):
            rlo = 32 if m == 0 else 0
            np_ = 128 - rlo
            for n in range(VL // 500):
                fp = fcps.tile([128, 512], f32, tag="fp")
                for c in range(5):
                    nc.tensor.matmul(
                        fp[0:np_, 0:500],
                        WO[c][:, 128 * m + rlo:128 * m + 128],
                        FC[c][:, 500 * n:500 * n + 500],
                        start=(c == 0), stop=(c == 4))
                lo = lout.tile([128, 500], f16, tag="lo")
                nc.vector.tensor_copy(lo[0:np_, :], fp[0:np_, 0:500])
                dst = out[4 * m + (1 if m == 0 else 0):4 * m + 4, :,
                          500 * n:500 * n + 500]
                if n % 2 == 0:
                    nc.sync.dma_start(dst, lo[0:np_, :])
                else:
                    nc.gpsimd.dma_start(dst, lo[0:np_, :])

        # ---------- initial state ----------
        sp = fcps.tile([128, 512], f32, tag="fp")
        for ci in range(3):
            nc.tensor.matmul(sp[0:64, 0:LO_W], FEAT[ci], SZT[ci][:, 0:LO_W],
                             start=(ci == 0), stop=(ci == 2),
                             tile_position=(0, 0))
            nc.tensor.matmul(sp[64:128, 0:HI_W], FEAT[ci], SZT[ci][:, LO_W:H],
                             start=(ci == 0), stop=(ci == 2),
                             tile_position=(0, 64))
        CS = states.tile([128, LO_W], f16, tag="cs16")
        nc.vector.memset(CS[:], 0.0)
        nc.vector.tensor_copy(CS[0:64, 0:LO_W], sp[0:64, 0:LO_W])
        nc.vector.tensor_copy(CS[64:128, 0:HI_W], sp[64:128, 0:HI_W])
        HS = states.tile([128, LO_W], f16, tag="hs16")
        nc.vector.memset(HS[:], 0.0)
        CUN = states.tile([128, 32], f16, tag="cun")
        nc.vector.memset(CUN[:], 0.0)
        HUN = states.tile([128, 32], f16, tag="hun")
        nc.vector.memset(HUN[:], 0.0)
        XTS = emit_transposes(CS, HS, CUN, HUN)

        # ---------- recurrence ----------
        pending_ft = None
        for s in range(NSTEP):
            def xchunk(c, _xts=XTS, _s=s):
                if c == 0:
                    return WT0[:, 64 * _s:64 * _s + 64]
                if c == 1:
                    return WT1[:, 64 * _s:64 * _s + 64]
                r = _chunk_rows(c)
                return _xts[0:r, 64 * (c - 2):64 * (c - 2) + 64]

            gps = [pool.tile([128, 512], f32, tag=f"p{k}", name=f"p{k}")
                   for k, pool in enumerate((gpsA, gpsB, gpsC))]
            for c in range(NCHUNK):
                xc = xchunk(c)
                for k, pk in enumerate(gps):
                    a0, a1 = BANK_A[k]
                    b0, b1 = BANK_B[k]
                    nc.tensor.matmul(pk[0:64, 0:a1 - a0], xc,
                                     WA[c][:, a0:a1],
                                     start=(c == 0), stop=(c == NCHUNK - 1),
                                     tile_position=(0, 0))
                    nc.tensor.matmul(pk[64:128, 0:b1 - b0], xc,
                                     WA[c][:, b0:b1],
                                     start=(c == 0), stop=(c == NCHUNK - 1),
                                     tile_position=(0, 64))
            P0, P1, P2 = gps

            # deferred make_ft matmuls + WO writeback from the previous step
            if pending_ft is not None:
                pending_ft()
                pending_ft = None

            act = nc.scalar.activation
            SIG, TANH = AF.Sigmoid, AF.Tanh
            # gate layout [g | i | f | o]: 4 full-width jobs + 2 u jobs
            act(GT[:, 0:320], P0[:, 0:320], TANH)        # g (both halves)
            act(GT[:, 320:512], P0[:, 320:512], SIG)     # i head
            act(GT[:, 512:1024], P1[:, 0:512], SIG)      # i tail, f, o head
            act(GT[:, 1024:1280], P2[:, 0:256], SIG)     # o tail
            act(GUS[64:128, :], P2[64:128, 256:331], SIG)
            act(GUT[64:128, :], P2[64:128, 331:356], TANH)

            CSn = states.tile([128, LO_W], f16, tag="cs16")
            nc.vector.tensor_mul(T1[:], GT[:, 640:960], CS[:])
            nc.vector.tensor_mul(T2[:], GT[:, 320:640], GT[:, 0:320])
            nc.vector.tensor_add(CSn[:], T1[:], T2[:])
            act(TC[:], CSn[:], TANH)
            HSn = states.tile([128, LO_W], f16, tag="hs16")
            nc.vector.tensor_mul(HSn[:], GT[:, 960:1280], TC[:])

            CUNn = states.tile([128, 32], f16, tag="cun")
            nc.vector.tensor_mul(T1U[64:128, :], GUS[64:128, 25:50],
                                 CUN[64:128, 0:25])
            nc.vector.tensor_mul(T2U[64:128, :], GUS[64:128, 0:25],
                                 GUT[64:128, :])
            nc.vector.tensor_add(CUNn[64:128, 0:25], T1U[64:128, :],
                                 T2U[64:128, :])
            nc.vector.memset(CUNn[64:128, 25:32], 0.0)
            act(TCU[64:128, :], CUNn[64:128, 0:25], TANH)
            HUNn = states.tile([128, 32], f16, tag="hun")
            nc.vector.tensor_mul(HUNn[64:128, 0:25], GUS[64:128, 50:75],
                                 TCU[64:128, :])
            nc.vector.memset(HUNn[64:128, 25:32], 0.0)

            XTSn = emit_transposes(CSn, HSn, CUNn, HUNn)

            # ---- make_ft (in-step part: ut matmul + DRAM regroup reads) ----
            nc.vector.tensor_copy(HUT[0:32, :], XTSn[32:64, 640:704])
            up = fcps.tile([128, 512], f32, tag="fp")
            nc.tensor.matmul(up[0:64, 0:LO_W], HUT[:], WUT[:, 0:LO_W],
                             start=True, stop=True, tile_position=(0, 0))
            nc.tensor.matmul(up[64:128, 0:HI_W], HUT[:], WUT[:, LO_W:H],
                             start=True, stop=True, tile_position=(0, 64))
            UT = ft_pool.tile([128, LO_W], f16, tag="ut")
            nc.vector.tensor_copy(UT[:], up[:, 0:LO_W])

            hsd = dram.tile([640, 64], f16, tag="hsd")
            for c in range(5):
                nc.sync.dma_start(hsd[128 * c:128 * c + 128, :],
                                  XTSn[:, 64 * (5 + c):64 * (5 + c) + 64])
            utd = dram.tile([BL, H], f16, tag="utd")
            nc.sync.dma_start(utd[:, 0:LO_W], UT[0:BL, 0:LO_W])
            nc.sync.dma_start(utd[:, LO_W:H], UT[64:64 + BL, 0:HI_W])
            # M2T layout [j, i*BL + b] (b contiguous to match hsd's layout)
            M2T = ft_pool.tile([D, BL * D], f16, tag="m2t")
            nc.sync.dma_start(
                M2T[:].rearrange("p (i b) -> p i b", b=BL),
                bass.AP(hsd.tensor, 0, [[1600, D], [64, D], [1, BL]]))
            UTT = ft_pool.tile([D, BL * D], f16, tag="utt")
            nc.sync.dma_start(
                UTT[:], bass.AP(utd.tensor, 0, [[D, D], [H, BL], [1, D]]))

            def make_ft_tail(M2T=M2T, UTT=UTT, tcol=s + 1):
                M2Tb = M2T[:].rearrange("p (i b) -> p b i", b=BL)
                ttp = fcps.tile([128, 512], f32, tag="fp")
                for w in range(8):
                    for jq in range(4):
                        b = 8 * jq + w
                        nc.tensor.matmul(ttp[32 * jq:32 * jq + D,
                                             D * w:D * w + D],
                                         M2Tb[:, b, :],
                                         UTT[:, D * b:D * b + D],
                                         start=True, stop=True,
                                         tile_position=(0, 32 * jq))
                TTS = ft_pool.tile([D, BL * D], f16, tag="tts")
                for jq in range(4):
                    dst = TTS[:].rearrange("p (i b) -> p i b", i=D)[
                        :, :, 8 * jq:8 * jq + 8]
                    src = ttp[32 * jq:32 * jq + D, 0:200].rearrange(
                        "p (w i) -> p i w", w=8)
                    nc.vector.tensor_copy(dst, src)
                ttd = dram.tile([D, BL * D], f16, tag="ttd")
                nc.sync.dma_start(ttd[:], TTS[:])
                for c in range(5):
                    nc.gpsimd.dma_start(
                        WO[c][0:125, BL * tcol:BL * tcol + BL],
                        bass.AP(ttd.tensor, 5 * c * BL * D,
                                [[BL * D, 5], [BL, D], [1, BL]]))

            pending_ft = make_ft_tail
            CS, HS, CUN, HUN, XTS = CSn, HSn, CUNn, HUNn, XTSn

            if s >= 3 and (s - 3) % 4 == 0:
                fc_mchunk((s - 3) // 4)

        pending_ft()
        fc_mchunk(5)

    nc.compile()
    return nc


def _host_prep(inputs):
    f32 = lambda k: np.asarray(inputs[k], dtype=np.float32)
    features = f32("features")
    captions = np.asarray(inputs["captions"]).astype(np.int64)
    embed = f32("embed_table")
    WihS, WhhS = f32("WihS"), f32("WhhS")
    bihS, bhhS = f32("bihS"), f32("bhhS")
    WihU, WhhU = f32("WihU"), f32("WhhU")
    bihU, bhhU = f32("bihU"), f32("bhhU")
    fcW, fcb = f32("fcW"), f32("fcb")
    szW, szb = f32("szW"), f32("szb")
    wuW, wub = f32("wuW"), f32("wub")

    w_base = np.zeros((XROWS, 2600), np.float32)
    WihS_T, WihU_T = WihS.T, WihU.T
    w_base[0:256, :2500] = WihS_T[25:281]
    w_base[0:256, 2500:] = WihU_T[25:281]
    w_base[256:881, :2500] = WihS_T[281:906]
    w_base[256:881, 2500:] = WihU_T[281:906]
    w_base[896:1521, :2500] = WhhS.T
    w_base[1536:1561, :2500] = WihS_T[0:25]
    w_base[1536:1561, 2500:] = WihU_T[0:25]
    w_base[1568:1593, 2500:] = WhhU.T
    w_base[1600, :2500] = bihS + bhhS
    w_base[1600, 2500:] = bihU + bhhU

    # gate-column scatter: order [g, i, f, o], lo/hi fold (hi padded to 320),
    # then u-gates [iu, fu, ou, gu]. Original S-gate order is [i, f, g, o].
    w_all = np.zeros((XROWS, GCOLS), np.float32)
    OLDQ = [2, 0, 1, 3]          # new gate slot -> original gate index
    for q in range(4):
        w_all[:, q * LO_W:(q + 1) * LO_W] = \
            w_base[:, OLDQ[q] * H:OLDQ[q] * H + LO_W]
        w_all[:, 1280 + q * LO_W:1280 + q * LO_W + HI_W] = \
            w_base[:, OLDQ[q] * H + LO_W:(OLDQ[q] + 1) * H]
    w_all[:, 2560:2585] = w_base[:, 2500:2525]   # iu
    w_all[:, 2585:2610] = w_base[:, 2525:2550]   # fu
    w_all[:, 2610:2635] = w_base[:, 2575:2600]   # ou
    w_all[:, 2635:2660] = w_base[:, 2550:2575]   # gu
    w_all = np.ascontiguousarray(w_all).astype(F16)

    fcW_perm = np.ascontiguousarray(
        fcW.reshape(V, D, D).transpose(0, 2, 1).reshape(V, H))
    wuW_perm = np.ascontiguousarray(
        wuW.reshape(D, D, D).transpose(1, 0, 2).reshape(H, D))
    wub_perm = np.ascontiguousarray(wub.reshape(D, D).T.reshape(H))
    wut = np.zeros((34, H), np.float32)
    wut[0:25] = wuW_perm.T
    wut[32] = wub_perm
    wut = wut.astype(F16)

    szt = np.concatenate([szW.T, szb[None, :]], 0).astype(F16)
    emb16 = embed.astype(F16)

    in_maps = []
    for bh in range(NB):
        rot = np.roll(np.arange(B), -BL * bh)
        feat_r = np.concatenate([features[rot].T,
                                 np.ones((1, B), np.float32)], 0).astype(F16)
        cap_r = captions[rot]
        idx = np.zeros((NSTEP, B), np.int64)
        for s in range(1, NSTEP):
            idx[s] = cap_r[:, s]
        wt = np.ascontiguousarray(emb16[idx.reshape(-1)].T)  # [E, 64*NSTEP]
        for vq in range(NV):
            fcq = np.concatenate(
                [fcW_perm.T[:, VL * vq:VL * vq + VL],
                 fcb[None, VL * vq:VL * vq + VL]], 0).astype(F16)
            in_maps.append({
                "w_all": w_all, "fcq": np.ascontiguousarray(fcq),
                "wut": wut, "feat": feat_r, "szt": szt,
                "wtin": wt, "onesf": np.ones((1, T * BL), F16),
                "iden": np.eye(128, dtype=F16),
            })
    return in_maps


def kernel(**inputs):
    from concourse.bass_utils import run_bass_kernel_spmd

    if "prog" not in _COMPILED:
        _COMPILED["prog"] = _build_program()
    nc = _COMPILED["prog"]

    in_maps = _host_prep(inputs)
    res = run_bass_kernel_spmd(nc, in_maps, list(range(NCORES)))

    fcb = np.asarray(inputs["fcb"], dtype=np.float32)
    out = np.zeros((T, B, 1, V), np.float32)
    ci = 0
    for bh in range(NB):
        for vq in range(NV):
            o = np.asarray(res.results[ci]["out"]).astype(np.float32)
            out[:, BL * bh:BL * bh + BL, 0, VL * vq:VL * vq + VL] = o
            ci += 1
    out[0, :, 0, :] = fcb[None, :]
    return out
